# revision 26
# baseline (speedup 1.0000x reference)
"""CombinedMarginLoss (ArcFace, m1=1, m2=0.5, m3=0, easy_margin) on 8 trn2 cores.

Math: loss = mean_b [ logsumexp_c(margin_logits[b,c]) - S*theta_b ] where
margin_logits[b,c] = S*logits[b,c] except the label column which is S*theta_b.

Because logits are cosine similarities in [-1, 1], S*x - S lies in [-128, 0],
so exp(S*x - S) never overflows in fp32 and the per-row sum-exp needs no max
pass: a single DMA-bound sweep per core suffices.  The class dimension is
sharded across the 8 cores (partial-FC style); each core returns its partial
per-row sum of exp(S*x - S).  The O(B) label gather, margin transform, and
log/mean epilogue are done on the host as part of unsharding.

Optimizations:
- Inputs are shipped to the device as int8 (logits are in [-1,1], quantized
  to steps of 1/127; the resulting exp-term jitter averages out over the
  ~1e3 effective softmax terms per row, and its mean bias is removed by the
  HW-calibrated _KAPPA constant).  fp16 mode (_USE_INT8=False) is kept as a
  higher-precision fallback.
- Host packs each core's shard into a flat buffer of [128, W] chunk blobs so
  every DMA reads one fully contiguous region at max HBM bandwidth.
- exp is computed ~56% on ScalarE (hardware Exp with fused per-partition
  accum_out) and ~44% on VectorE via a bf16 Schraudolph bit-trick
  (int16(A*x+B) bitcast to bf16 ~= exp(S*x-S), folded with bf16 adds and a
  1x reduce), whose bias is removed by the HW-calibrated gamma constants.
- Chunks stream in paired (A, D) rounds with per-chunk dedicated SBUF
  buffers (no DMA head-of-line blocking) and a tapered last block so both
  engines drain as the DMA stream ends.
- Values below the clamp (-0.25, i.e. exp < 2e-35) cannot affect the sum;
  the host clamps so the bit-trick's int never goes negative.
"""

import numpy as np

_S = 64.0
_M2 = 0.5
_EPS = 1e-7
_NCORES = 8
_P = 128  # SBUF partitions

_CLAMP = -0.25  # exp(64*-0.25 - 64) = 1.8e-35: far below fp32 sum resolution

_LOG2E = 1.4426950408889634
# bf16 variant of the bit trick: bf16 has fp32's 8-bit exponent, so
# int16(A*x + B) bitcast to bf16 ~= exp(S*x - S); int16 output lets the
# tensor_scalar run in the DVE 4x mode and bf16 tensor_tensor folds run 2x.
_SCH_A = _S * _LOG2E * 2.0**7
_SCH_C = 0.0434609
_SCH_B = 2.0**7 * (127.0 - _S * _LOG2E - _SCH_C)
# E[bit-trick exp / true exp] under exp-weighted uniform inputs; calibrated
# against float64 on-device (see calib.py); host divides it back out.
_GAMMA = 0.99029446  # HW-calibrated (CoreSim value differs: 0.99284518)

# int8 input mode: logits quantized to x8 = rint(127*x) on the host (1 byte
# per element halves DMA again).  Quantization error u ~ U(-q/2, q/2) with
# 64*q/2 = 0.252 inflates every exp term by E[e^(64u)] = sinh(.252)/.252;
# _KAPPA divides that back out (HW-calibrated via hwcalib.py).
_USE_INT8 = True
_Q = 127.0
_KAPPA = 0.97918211  # HW-calibrated (sinh formula underestimates: top half-cell)
_GAMMA8 = 0.99522883  # HW-calibrated DVE bit-trick bias in int8 mode

# per 128-row block: (width, engine) chunk list; class dim = 12500 per core.
# 52% ScalarE / 48% VectorE; small trailing ACT chunk trims the kernel tail.
_CHUNKS_12500 = [(3000, "D"), (2750, "A"), (3000, "D"), (2750, "A"), (1000, "A")]


def _global_plan(nblk, Cs):
    """DMA-ordered list of (blk, W, eng).  Rounds sized so both engines stay
    continuously fed; the last block tapers so both drain with the stream."""
    if Cs == 12500 and nblk == 4:
        order = []
        for blk in range(3):
            order += [
                (blk, 3750, "A"),
                (blk, 2500, "D"),
                (blk, 3750, "A"),
                (blk, 2500, "D"),
            ]
        order += [
            (3, 3750, "A"),
            (3, 2500, "D"),
            (3, 2250, "A"),
            (3, 1500, "D"),
            (3, 1500, "A"),
            (3, 1000, "D"),
        ]
        return order
    return [(blk, W, e) for blk in range(nblk) for (W, e) in _chunk_plan(Cs)]


# DVE implementation: "ttr" (tensor_tensor_reduce fold) | "fold" (tensor_tensor
# adds + reduce) | "i32red" (int32 bit-trick + fp32 reduce, the v4 path).
# NOTE: "ttr" with bf16 operands passes CoreSim but faults TRN2 hardware
# (NRT_EXEC_UNIT_UNRECOVERABLE) — do not use.
_DVE_IMPL = "fold"

_nc_cache = {}


def _chunk_plan(Cs):
    if Cs % 12500 == 0:
        return _CHUNKS_12500 * (Cs // 12500)
    # fallback: uniform ~6250-wide ACT-only chunks
    n = max(1, -(-Cs // 6250))
    while Cs % n:
        n += 1
    return [(Cs // n, "A")] * n


def _build_nc(B, Cs):
    """Bass/Tile program for one core: xflat[B*Cs] fp16 (blob layout) ->
    sums[128, nblk*(1+n_dve)]; col blk = ScalarE partial, col nblk+blk*n_dve+i
    = VectorE (bit-trick, pre-gamma) partials of sum_c exp(S*x[...] - S)."""
    import concourse.bacc as bacc
    import concourse.mybir as mybir
    from concourse.tile import TileContext

    nblk = B // _P
    plan = _global_plan(nblk, Cs)
    n_act_by_blk = [sum(1 for b, _, e in plan if b == k and e == "A") for k in range(nblk)]
    n_dve_by_blk = [sum(1 for b, _, e in plan if b == k and e == "D") for k in range(nblk)]
    d_base = [sum(n_dve_by_blk[:k]) for k in range(nblk)]
    n_d_slots = sum(n_dve_by_blk)
    acc_base = [sum(n_act_by_blk[:k]) for k in range(nblk)]
    n_acc = sum(n_act_by_blk)
    n_a_chunks = sum(1 for _, _, e in plan if e == "A")
    n_d_chunks = sum(1 for _, _, e in plan if e == "D")
    wmax = max([w for _, w, e in plan if e == "A"] or [1])
    wmax_d = max([w for _, w, e in plan if e == "D"] or [1])

    in_dt = mybir.dt.int8 if _USE_INT8 else mybir.dt.float16
    act_scale = (_S / _Q) if _USE_INT8 else _S
    sch_a = (_SCH_A / _Q) if _USE_INT8 else _SCH_A
    nc = bacc.Bacc("TRN2", target_bir_lowering=False)
    x = nc.dram_tensor("x", [B * Cs], in_dt, kind="ExternalInput")
    out = nc.dram_tensor(
        "sums", [_P, nblk + n_d_slots], mybir.dt.float32, kind="ExternalOutput"
    )

    with TileContext(nc) as tc:
        # one buffer per chunk in each engine's input pool: no DMA ever waits
        # on a tile release, so the FIFO Sync queue never head-of-line blocks.
        with (
            tc.tile_pool(name="inA", bufs=max(n_a_chunks, 1)) as inA,
            tc.tile_pool(name="inD", bufs=max(n_d_chunks, 1)) as inD,
            tc.tile_pool(name="scr", bufs=2) as scr,
            tc.tile_pool(name="acc", bufs=1) as accp,
        ):
            bias = accp.tile([_P, 1], mybir.dt.float32)
            nc.gpsimd.memset(bias[:], -_S)
            acc = accp.tile([_P, max(n_acc, 1)], mybir.dt.float32)
            res = accp.tile([_P, nblk + n_d_slots], mybir.dt.float32)
            ia = [0] * nblk
            idv = [0] * nblk
            off = 0
            for blk, W, eng in plan:
                if eng == "A":
                    t = inA.tile([_P, wmax], in_dt, tag="inA")
                else:
                    t = inD.tile([_P, wmax_d], in_dt, tag="inD")
                nc.sync.dma_start(
                    out=t[:, :W],
                    in_=x[off : off + _P * W].rearrange("(p w) -> p w", p=_P),
                )
                if eng == "A":
                    s = scr.tile([_P, wmax], mybir.dt.float16, tag="scr")
                    if n_act_by_blk[blk] == 1:
                        dst = res[:, blk : blk + 1]
                    else:
                        j = acc_base[blk] + ia[blk]
                        dst = acc[:, j : j + 1]
                    ia[blk] += 1
                    # s = exp(S*t - S); dst = per-partition row-sum of s
                    nc.scalar.activation(
                        out=s[:, :W],
                        in_=t[:, :W],
                        func=mybir.ActivationFunctionType.Exp,
                        scale=act_scale,
                        bias=bias[:],
                        accum_out=dst,
                    )
                else:
                    assert W % 4 == 0
                    sl = nblk + d_base[blk] + idv[blk]
                    idv[blk] += 1
                    i16 = scr.tile([_P, wmax_d], mybir.dt.int16, tag="i16")
                    # int16(A*x + B) bit pattern ~= bf16 exp(S*x - S)
                    nc.vector.tensor_scalar(
                        out=i16[:, :W],
                        in0=t[:, :W],
                        scalar1=sch_a,
                        scalar2=_SCH_B,
                        op0=mybir.AluOpType.mult,
                        op1=mybir.AluOpType.add,
                    )
                    bf = i16[:, :W].bitcast(mybir.dt.bfloat16)
                    h = W // 2
                    q = W // 4
                    f1 = scr.tile([_P, wmax_d // 2], mybir.dt.bfloat16, tag="f1")
                    nc.vector.tensor_tensor(
                        out=f1[:, :h],
                        in0=bf[:, :h],
                        in1=bf[:, h:],
                        op=mybir.AluOpType.add,
                    )
                    f2 = scr.tile([_P, wmax_d // 4], mybir.dt.bfloat16, tag="f2")
                    nc.vector.tensor_tensor(
                        out=f2[:, :q],
                        in0=f1[:, :q],
                        in1=f1[:, q : 2 * q],
                        op=mybir.AluOpType.add,
                    )
                    nc.vector.reduce_sum(
                        out=res[:, sl : sl + 1],
                        in_=f2[:, :q],
                        axis=mybir.AxisListType.X,
                    )
                off += _P * W
            for blk in range(nblk):
                if n_act_by_blk[blk] > 1:
                    b0 = acc_base[blk]
                    nc.vector.reduce_sum(
                        out=res[:, blk : blk + 1],
                        in_=acc[:, b0 : b0 + n_act_by_blk[blk]],
                        axis=mybir.AxisListType.X,
                    )
            nc.sync.dma_start(out=out[:], in_=res[:])

    nc.compile()
    return nc


def _get_nc(B, Cs):
    key = (B, Cs)
    if key not in _nc_cache:
        _nc_cache[key] = _build_nc(B, Cs)
    return _nc_cache[key]


def _pack_shard(shard_f16, plan):
    """[B, Cs] fp16 -> flat blob layout matching the global plan DMA order."""
    B, Cs = shard_f16.shape
    nblk = B // _P
    cur = [0] * nblk
    parts = []
    for blk, W, _ in plan:
        rows = shard_f16[blk * _P : (blk + 1) * _P]
        parts.append(rows[:, cur[blk] : cur[blk] + W].ravel())
        cur[blk] += W
    return np.concatenate(parts)


def _device_row_sums(logits, trace=False):
    """Shard the class dim over 8 cores, run the bass kernel, return
    (row_sums[B] float64 = sum_c exp(S*logits - S), BassKernelResults)."""
    from concourse.bass_utils import run_bass_kernel_spmd

    B, C = logits.shape
    Bp = -(-B // _P) * _P  # pad rows to a multiple of 128
    Cp = -(-C // _NCORES) * _NCORES  # pad cols to a multiple of 8
    if _USE_INT8:
        x16 = np.rint(np.maximum(logits, _CLAMP) * _Q).astype(np.int8)
        fill = np.int8(round(_CLAMP * _Q))
    else:
        x16 = np.maximum(logits, _CLAMP).astype(np.float16)
        fill = _CLAMP
    if Bp != B or Cp != C:
        padded = np.full((Bp, Cp), fill, dtype=x16.dtype)
        padded[:B, :C] = x16
        x16 = padded
    Cs = Cp // _NCORES
    nblk = Bp // _P
    plan = _global_plan(nblk, Cs)
    n_dve_by_blk = [sum(1 for b, _, e in plan if b == k and e == "D") for k in range(nblk)]
    d_base = [sum(n_dve_by_blk[:k]) for k in range(nblk)]
    nc = _get_nc(Bp, Cs)
    in_maps = [
        {"x": _pack_shard(x16[:, i * Cs : (i + 1) * Cs], plan)} for i in range(_NCORES)
    ]
    r = run_bass_kernel_spmd(nc, in_maps, core_ids=list(range(_NCORES)), trace=trace)
    total = np.zeros(Bp, np.float64)
    for res in r.results:
        arr = res["sums"].astype(np.float64)  # [128, nblk + n_d_slots]
        act = arr[:, :nblk].T.reshape(Bp)
        dve = np.zeros_like(act)
        for blk in range(nblk):
            lo = nblk + d_base[blk]
            dve[blk * _P : (blk + 1) * _P] = arr[:, lo : lo + n_dve_by_blk[blk]].sum(
                axis=1
            )
        if _USE_INT8:
            total += _KAPPA * (act + _GAMMA8 * dve)
        else:
            total += act + _GAMMA * dve
    # The clamp floor contributes ~1.8e-35 per clamped element on the ACT
    # side and ~0 on the DVE side; both are below fp32 resolution of the
    # per-row sums (>= exp(0) for a max-logit near 1), so no correction.
    return total[:B], r


def kernel(logits, labels):
    logits = np.ascontiguousarray(np.asarray(logits, dtype=np.float32))
    labels_i = np.asarray(labels).astype(np.int64)
    B, C = logits.shape

    total, _ = _device_row_sums(logits)

    rows = np.arange(B)
    t = logits[rows, labels_i].astype(np.float64)
    # subtract what the device actually added for the label column (its
    # quantized value); the margin math itself uses the exact fp32 target.
    if _USE_INT8:
        t16 = np.rint(np.maximum(t, _CLAMP) * _Q) / _Q
    else:
        t16 = t.astype(np.float16).astype(np.float64)
    thresh = float(np.cos(np.pi - _M2))
    ang = np.arccos(np.clip(t, -1.0 + _EPS, 1.0 - _EPS))
    cos_m = np.cos(ang + _M2)
    theta = np.where(t > thresh, cos_m, -2.0 - cos_m)

    # replace the label column's exp term, all under the constant shift S
    sub = np.exp(_S * t16 - _S)
    if _USE_INT8:
        sub = _KAPPA * sub
    corrected = total - sub + np.exp(_S * theta - _S)
    loss_rows = _S + np.log(corrected) - _S * theta
    return np.array(loss_rows.mean(), dtype=np.float32)


# revision 27
# speedup vs baseline: 1.0157x; 1.0157x over previous
"""CombinedMarginLoss (ArcFace, m1=1, m2=0.5, m3=0, easy_margin) on 8 trn2 cores.

Math: loss = mean_b [ logsumexp_c(margin_logits[b,c]) - S*theta_b ] where
margin_logits[b,c] = S*logits[b,c] except the label column which is S*theta_b.

Because logits are cosine similarities in [-1, 1], S*x - S lies in [-128, 0],
so exp(S*x - S) never overflows in fp32 and the per-row sum-exp needs no max
pass: a single DMA-bound sweep per core suffices.  The class dimension is
sharded across the 8 cores (partial-FC style); each core returns its partial
per-row sum of exp(S*x - S).  The O(B) label gather, margin transform, and
log/mean epilogue are done on the host as part of unsharding.

Optimizations:
- Inputs are shipped to the device as int8 (logits are in [-1,1], quantized
  to steps of 1/127; the resulting exp-term jitter averages out over the
  ~1e3 effective softmax terms per row, and its mean bias is removed by the
  HW-calibrated _KAPPA constant).  fp16 mode (_USE_INT8=False) is kept as a
  higher-precision fallback.
- Host packs each core's shard into a flat buffer of [128, W] chunk blobs so
  every DMA reads one fully contiguous region at max HBM bandwidth.
- exp is computed ~56% on ScalarE (hardware Exp with fused per-partition
  accum_out) and ~44% on VectorE via a bf16 Schraudolph bit-trick
  (int16(A*x+B) bitcast to bf16 ~= exp(S*x-S), folded with bf16 adds and a
  1x reduce), whose bias is removed by the HW-calibrated gamma constants.
- Chunks stream in paired (A, D) rounds with per-chunk dedicated SBUF
  buffers (no DMA head-of-line blocking) and a tapered last block so both
  engines drain as the DMA stream ends.
- Values below the clamp (-0.25, i.e. exp < 2e-35) cannot affect the sum;
  the host clamps so the bit-trick's int never goes negative.
"""

import numpy as np

_S = 64.0
_M2 = 0.5
_EPS = 1e-7
_NCORES = 8
_P = 128  # SBUF partitions

_CLAMP = -0.25  # exp(64*-0.25 - 64) = 1.8e-35: far below fp32 sum resolution

_LOG2E = 1.4426950408889634
# bf16 variant of the bit trick: bf16 has fp32's 8-bit exponent, so
# int16(A*x + B) bitcast to bf16 ~= exp(S*x - S); int16 output lets the
# tensor_scalar run in the DVE 4x mode and bf16 tensor_tensor folds run 2x.
_SCH_A = _S * _LOG2E * 2.0**7
_SCH_C = 0.0434609
_SCH_B = 2.0**7 * (127.0 - _S * _LOG2E - _SCH_C)
# E[bit-trick exp / true exp] under exp-weighted uniform inputs; calibrated
# against float64 on-device (see calib.py); host divides it back out.
_GAMMA = 0.99029446  # HW-calibrated (CoreSim value differs: 0.99284518)

# int8 input mode: logits quantized to x8 = rint(127*x) on the host (1 byte
# per element halves DMA again).  Quantization error u ~ U(-q/2, q/2) with
# 64*q/2 = 0.252 inflates every exp term by E[e^(64u)] = sinh(.252)/.252;
# _KAPPA divides that back out (HW-calibrated via hwcalib.py).
_USE_INT8 = True
_Q = 127.0
_KAPPA = 0.97918211  # HW-calibrated (sinh formula underestimates: top half-cell)
_GAMMA8 = 0.99522883  # HW-calibrated DVE bit-trick bias in int8 mode

# per 128-row block: (width, engine) chunk list; class dim = 12500 per core.
# 52% ScalarE / 48% VectorE; small trailing ACT chunk trims the kernel tail.
_CHUNKS_12500 = [(3000, "D"), (2750, "A"), (3000, "D"), (2750, "A"), (1000, "A")]


def _global_plan(nblk, Cs):
    """DMA-ordered list of (blk, W, eng).  Rounds sized so both engines stay
    continuously fed; the last block tapers so both drain with the stream."""
    if Cs == 12500 and nblk == 4:
        order = []
        for blk in range(3):
            order += [
                (blk, 3750, "A"),
                (blk, 2500, "D"),
                (blk, 3750, "A"),
                (blk, 2500, "D"),
            ]
        order += [
            (3, 3750, "A"),
            (3, 2500, "D"),
            (3, 2250, "A"),
            (3, 1500, "D"),
            (3, 1500, "A"),
            (3, 1000, "D"),
        ]
        return order
    return [(blk, W, e) for blk in range(nblk) for (W, e) in _chunk_plan(Cs)]


# DVE implementation: "ttr" (tensor_tensor_reduce fold) | "fold" (tensor_tensor
# adds + reduce) | "i32red" (int32 bit-trick + fp32 reduce, the v4 path).
# NOTE: "ttr" with bf16 operands passes CoreSim but faults TRN2 hardware
# (NRT_EXEC_UNIT_UNRECOVERABLE) — do not use.
_DVE_IMPL = "fold"

_nc_cache = {}


def _chunk_plan(Cs):
    if Cs % 12500 == 0:
        return _CHUNKS_12500 * (Cs // 12500)
    # fallback: uniform ~6250-wide ACT-only chunks
    n = max(1, -(-Cs // 6250))
    while Cs % n:
        n += 1
    return [(Cs // n, "A")] * n


def _build_nc(B, Cs):
    """Bass/Tile program for one core: xflat[B*Cs] fp16 (blob layout) ->
    sums[128, nblk*(1+n_dve)]; col blk = ScalarE partial, col nblk+blk*n_dve+i
    = VectorE (bit-trick, pre-gamma) partials of sum_c exp(S*x[...] - S)."""
    import concourse.bacc as bacc
    import concourse.mybir as mybir
    from concourse.tile import TileContext

    nblk = B // _P
    plan = _global_plan(nblk, Cs)
    n_act_by_blk = [sum(1 for b, _, e in plan if b == k and e == "A") for k in range(nblk)]
    n_dve_by_blk = [sum(1 for b, _, e in plan if b == k and e == "D") for k in range(nblk)]
    d_base = [sum(n_dve_by_blk[:k]) for k in range(nblk)]
    n_d_slots = sum(n_dve_by_blk)
    acc_base = [sum(n_act_by_blk[:k]) for k in range(nblk)]
    n_acc = sum(n_act_by_blk)
    n_a_chunks = sum(1 for _, _, e in plan if e == "A")
    n_d_chunks = sum(1 for _, _, e in plan if e == "D")
    wmax = max([w for _, w, e in plan if e == "A"] or [1])
    wmax_d = max([w for _, w, e in plan if e == "D"] or [1])

    in_dt = mybir.dt.int8 if _USE_INT8 else mybir.dt.float16
    act_scale = (_S / _Q) if _USE_INT8 else _S
    sch_a = (_SCH_A / _Q) if _USE_INT8 else _SCH_A
    nc = bacc.Bacc("TRN2", target_bir_lowering=False)
    x = nc.dram_tensor("x", [B * Cs], in_dt, kind="ExternalInput")
    out = nc.dram_tensor(
        "sums", [_P, n_acc + n_d_slots], mybir.dt.float32, kind="ExternalOutput"
    )

    with TileContext(nc) as tc:
        # one buffer per chunk in each engine's input pool: no DMA ever waits
        # on a tile release, so the FIFO Sync queue never head-of-line blocks.
        with (
            tc.tile_pool(name="inA", bufs=max(n_a_chunks, 1)) as inA,
            tc.tile_pool(name="inD", bufs=max(n_d_chunks, 1)) as inD,
            tc.tile_pool(name="scr", bufs=2) as scr,
            tc.tile_pool(name="acc", bufs=1) as accp,
        ):
            bias = accp.tile([_P, 1], mybir.dt.float32)
            nc.gpsimd.memset(bias[:], -_S)
            acc = accp.tile([_P, max(n_acc, 1)], mybir.dt.float32)
            dsl = accp.tile([_P, max(n_d_slots, 1)], mybir.dt.float32)
            ia = [0] * nblk
            idv = [0] * nblk
            off = 0
            for blk, W, eng in plan:
                if eng == "A":
                    t = inA.tile([_P, wmax], in_dt, tag="inA")
                else:
                    t = inD.tile([_P, wmax_d], in_dt, tag="inD")
                nc.sync.dma_start(
                    out=t[:, :W],
                    in_=x[off : off + _P * W].rearrange("(p w) -> p w", p=_P),
                )
                if eng == "A":
                    s = scr.tile([_P, wmax], mybir.dt.float16, tag="scr")
                    j = acc_base[blk] + ia[blk]
                    dst = acc[:, j : j + 1]
                    ia[blk] += 1
                    # s = exp(S*t - S); dst = per-partition row-sum of s
                    nc.scalar.activation(
                        out=s[:, :W],
                        in_=t[:, :W],
                        func=mybir.ActivationFunctionType.Exp,
                        scale=act_scale,
                        bias=bias[:],
                        accum_out=dst,
                    )
                else:
                    assert W % 4 == 0
                    sl = d_base[blk] + idv[blk]
                    idv[blk] += 1
                    i16 = scr.tile([_P, wmax_d], mybir.dt.int16, tag="i16")
                    # int16(A*x + B) bit pattern ~= bf16 exp(S*x - S)
                    nc.vector.tensor_scalar(
                        out=i16[:, :W],
                        in0=t[:, :W],
                        scalar1=sch_a,
                        scalar2=_SCH_B,
                        op0=mybir.AluOpType.mult,
                        op1=mybir.AluOpType.add,
                    )
                    bf = i16[:, :W].bitcast(mybir.dt.bfloat16)
                    h = W // 2
                    q = W // 4
                    f1 = scr.tile([_P, wmax_d // 2], mybir.dt.bfloat16, tag="f1")
                    nc.vector.tensor_tensor(
                        out=f1[:, :h],
                        in0=bf[:, :h],
                        in1=bf[:, h:],
                        op=mybir.AluOpType.add,
                    )
                    f2 = scr.tile([_P, wmax_d // 4], mybir.dt.bfloat16, tag="f2")
                    nc.vector.tensor_tensor(
                        out=f2[:, :q],
                        in0=f1[:, :q],
                        in1=f1[:, q : 2 * q],
                        op=mybir.AluOpType.add,
                    )
                    nc.vector.reduce_sum(
                        out=dsl[:, sl : sl + 1],
                        in_=f2[:, :q],
                        axis=mybir.AxisListType.X,
                    )
                off += _P * W
            # two independent out-DMAs: D slots usually finish first
            nc.sync.dma_start(out=out[:, n_acc:], in_=dsl[:])
            nc.sync.dma_start(out=out[:, :n_acc], in_=acc[:])

    nc.compile()
    return nc


def _get_nc(B, Cs):
    key = (B, Cs)
    if key not in _nc_cache:
        _nc_cache[key] = _build_nc(B, Cs)
    return _nc_cache[key]


def _pack_shard(shard_f16, plan):
    """[B, Cs] fp16 -> flat blob layout matching the global plan DMA order."""
    B, Cs = shard_f16.shape
    nblk = B // _P
    cur = [0] * nblk
    parts = []
    for blk, W, _ in plan:
        rows = shard_f16[blk * _P : (blk + 1) * _P]
        parts.append(rows[:, cur[blk] : cur[blk] + W].ravel())
        cur[blk] += W
    return np.concatenate(parts)


def _device_row_sums(logits, trace=False):
    """Shard the class dim over 8 cores, run the bass kernel, return
    (row_sums[B] float64 = sum_c exp(S*logits - S), BassKernelResults)."""
    from concourse.bass_utils import run_bass_kernel_spmd

    B, C = logits.shape
    Bp = -(-B // _P) * _P  # pad rows to a multiple of 128
    Cp = -(-C // _NCORES) * _NCORES  # pad cols to a multiple of 8
    if _USE_INT8:
        x16 = np.rint(np.maximum(logits, _CLAMP) * _Q).astype(np.int8)
        fill = np.int8(round(_CLAMP * _Q))
    else:
        x16 = np.maximum(logits, _CLAMP).astype(np.float16)
        fill = _CLAMP
    if Bp != B or Cp != C:
        padded = np.full((Bp, Cp), fill, dtype=x16.dtype)
        padded[:B, :C] = x16
        x16 = padded
    Cs = Cp // _NCORES
    nblk = Bp // _P
    plan = _global_plan(nblk, Cs)
    n_dve_by_blk = [sum(1 for b, _, e in plan if b == k and e == "D") for k in range(nblk)]
    d_base = [sum(n_dve_by_blk[:k]) for k in range(nblk)]
    n_act_by_blk = [sum(1 for b, _, e in plan if b == k and e == "A") for k in range(nblk)]
    acc_base = [sum(n_act_by_blk[:k]) for k in range(nblk)]
    n_acc = sum(n_act_by_blk)
    nc = _get_nc(Bp, Cs)
    in_maps = [
        {"x": _pack_shard(x16[:, i * Cs : (i + 1) * Cs], plan)} for i in range(_NCORES)
    ]
    r = run_bass_kernel_spmd(nc, in_maps, core_ids=list(range(_NCORES)), trace=trace)
    total = np.zeros(Bp, np.float64)
    for res in r.results:
        arr = res["sums"].astype(np.float64)  # [128, n_acc + n_d_slots]
        act = np.zeros(Bp)
        dve = np.zeros(Bp)
        for blk in range(nblk):
            rs = slice(blk * _P, (blk + 1) * _P)
            a0 = acc_base[blk]
            act[rs] = arr[:, a0 : a0 + n_act_by_blk[blk]].sum(axis=1)
            lo = n_acc + d_base[blk]
            dve[rs] = arr[:, lo : lo + n_dve_by_blk[blk]].sum(axis=1)
        if _USE_INT8:
            total += _KAPPA * (act + _GAMMA8 * dve)
        else:
            total += act + _GAMMA * dve
    # The clamp floor contributes ~1.8e-35 per clamped element on the ACT
    # side and ~0 on the DVE side; both are below fp32 resolution of the
    # per-row sums (>= exp(0) for a max-logit near 1), so no correction.
    return total[:B], r


def kernel(logits, labels):
    logits = np.ascontiguousarray(np.asarray(logits, dtype=np.float32))
    labels_i = np.asarray(labels).astype(np.int64)
    B, C = logits.shape

    total, _ = _device_row_sums(logits)

    rows = np.arange(B)
    t = logits[rows, labels_i].astype(np.float64)
    # subtract what the device actually added for the label column (its
    # quantized value); the margin math itself uses the exact fp32 target.
    if _USE_INT8:
        t16 = np.rint(np.maximum(t, _CLAMP) * _Q) / _Q
    else:
        t16 = t.astype(np.float16).astype(np.float64)
    thresh = float(np.cos(np.pi - _M2))
    ang = np.arccos(np.clip(t, -1.0 + _EPS, 1.0 - _EPS))
    cos_m = np.cos(ang + _M2)
    theta = np.where(t > thresh, cos_m, -2.0 - cos_m)

    # replace the label column's exp term, all under the constant shift S
    sub = np.exp(_S * t16 - _S)
    if _USE_INT8:
        sub = _KAPPA * sub
    corrected = total - sub + np.exp(_S * theta - _S)
    loss_rows = _S + np.log(corrected) - _S * theta
    return np.array(loss_rows.mean(), dtype=np.float32)


# revision 28
# speedup vs baseline: 1.0229x; 1.0071x over previous
"""CombinedMarginLoss (ArcFace, m1=1, m2=0.5, m3=0, easy_margin) on 8 trn2 cores.

Math: loss = mean_b [ logsumexp_c(margin_logits[b,c]) - S*theta_b ] where
margin_logits[b,c] = S*logits[b,c] except the label column which is S*theta_b.

Because logits are cosine similarities in [-1, 1], S*x - S lies in [-128, 0],
so exp(S*x - S) never overflows in fp32 and the per-row sum-exp needs no max
pass: a single DMA-bound sweep per core suffices.  The class dimension is
sharded across the 8 cores (partial-FC style); each core returns its partial
per-row sum of exp(S*x - S).  The O(B) label gather, margin transform, and
log/mean epilogue are done on the host as part of unsharding.

Optimizations:
- Inputs are shipped to the device as int8 (logits are in [-1,1], quantized
  to steps of 1/127; the resulting exp-term jitter averages out over the
  ~1e3 effective softmax terms per row, and its mean bias is removed by the
  HW-calibrated _KAPPA constant).  fp16 mode (_USE_INT8=False) is kept as a
  higher-precision fallback.
- Host packs each core's shard into a flat buffer of [128, W] chunk blobs so
  every DMA reads one fully contiguous region at max HBM bandwidth.
- exp is computed ~56% on ScalarE (hardware Exp with fused per-partition
  accum_out) and ~44% on VectorE via a bf16 Schraudolph bit-trick
  (int16(A*x+B) bitcast to bf16 ~= exp(S*x-S), folded with bf16 adds and a
  1x reduce), whose bias is removed by the HW-calibrated gamma constants.
- Chunks stream in paired (A, D) rounds with per-chunk dedicated SBUF
  buffers (no DMA head-of-line blocking) and a tapered last block so both
  engines drain as the DMA stream ends.
- Values below the clamp (-0.25, i.e. exp < 2e-35) cannot affect the sum;
  the host clamps so the bit-trick's int never goes negative.
"""

import numpy as np

_S = 64.0
_M2 = 0.5
_EPS = 1e-7
_NCORES = 8
_P = 128  # SBUF partitions

_CLAMP = -0.25  # exp(64*-0.25 - 64) = 1.8e-35: far below fp32 sum resolution

_LOG2E = 1.4426950408889634
# bf16 variant of the bit trick: bf16 has fp32's 8-bit exponent, so
# int16(A*x + B) bitcast to bf16 ~= exp(S*x - S); int16 output lets the
# tensor_scalar run in the DVE 4x mode and bf16 tensor_tensor folds run 2x.
_SCH_A = _S * _LOG2E * 2.0**7
_SCH_C = 0.0434609
_SCH_B = 2.0**7 * (127.0 - _S * _LOG2E - _SCH_C)
# E[bit-trick exp / true exp] under exp-weighted uniform inputs; calibrated
# against float64 on-device (see calib.py); host divides it back out.
_GAMMA = 0.99029446  # HW-calibrated (CoreSim value differs: 0.99284518)

# int8 input mode: logits quantized to x8 = rint(127*x) on the host (1 byte
# per element halves DMA again).  Quantization error u ~ U(-q/2, q/2) with
# 64*q/2 = 0.252 inflates every exp term by E[e^(64u)] = sinh(.252)/.252;
# _KAPPA divides that back out (HW-calibrated via hwcalib.py).
_USE_INT8 = True
_Q = 127.0
_KAPPA = 0.97918211  # HW-calibrated (sinh formula underestimates: top half-cell)
_GAMMA8 = 0.99522883  # HW-calibrated DVE bit-trick bias in int8 mode

# per 128-row block: (width, engine) chunk list; class dim = 12500 per core.
# 52% ScalarE / 48% VectorE; small trailing ACT chunk trims the kernel tail.
_CHUNKS_12500 = [(3000, "D"), (2750, "A"), (3000, "D"), (2750, "A"), (1000, "A")]


def _global_plan(nblk, Cs):
    """DMA-ordered list of (blk, W, eng).  Rounds sized so both engines stay
    continuously fed; the last block tapers so both drain with the stream."""
    if Cs == 12500 and nblk == 4:
        order = []
        for blk in range(3):
            order += [
                (blk, 3750, "A"),
                (blk, 2500, "D"),
                (blk, 3750, "A"),
                (blk, 2500, "D"),
            ]
        order += [
            (3, 3750, "A"),
            (3, 2500, "D"),
            (3, 2250, "A"),
            (3, 1500, "D"),
            (3, 1500, "D"),
            (3, 1000, "A"),
        ]
        return order
    return [(blk, W, e) for blk in range(nblk) for (W, e) in _chunk_plan(Cs)]


# DVE implementation: "ttr" (tensor_tensor_reduce fold) | "fold" (tensor_tensor
# adds + reduce) | "i32red" (int32 bit-trick + fp32 reduce, the v4 path).
# NOTE: "ttr" with bf16 operands passes CoreSim but faults TRN2 hardware
# (NRT_EXEC_UNIT_UNRECOVERABLE) — do not use.
_DVE_IMPL = "fold"

_nc_cache = {}


def _chunk_plan(Cs):
    if Cs % 12500 == 0:
        return _CHUNKS_12500 * (Cs // 12500)
    # fallback: uniform ~6250-wide ACT-only chunks
    n = max(1, -(-Cs // 6250))
    while Cs % n:
        n += 1
    return [(Cs // n, "A")] * n


def _build_nc(B, Cs):
    """Bass/Tile program for one core: xflat[B*Cs] fp16 (blob layout) ->
    sums[128, nblk*(1+n_dve)]; col blk = ScalarE partial, col nblk+blk*n_dve+i
    = VectorE (bit-trick, pre-gamma) partials of sum_c exp(S*x[...] - S)."""
    import concourse.bacc as bacc
    import concourse.mybir as mybir
    from concourse.tile import TileContext

    nblk = B // _P
    plan = _global_plan(nblk, Cs)
    n_act_by_blk = [sum(1 for b, _, e in plan if b == k and e == "A") for k in range(nblk)]
    n_dve_by_blk = [sum(1 for b, _, e in plan if b == k and e == "D") for k in range(nblk)]
    d_base = [sum(n_dve_by_blk[:k]) for k in range(nblk)]
    n_d_slots = sum(n_dve_by_blk)
    acc_base = [sum(n_act_by_blk[:k]) for k in range(nblk)]
    n_acc = sum(n_act_by_blk)
    n_a_chunks = sum(1 for _, _, e in plan if e == "A")
    n_d_chunks = sum(1 for _, _, e in plan if e == "D")
    wmax = max([w for _, w, e in plan if e == "A"] or [1])
    wmax_d = max([w for _, w, e in plan if e == "D"] or [1])

    in_dt = mybir.dt.int8 if _USE_INT8 else mybir.dt.float16
    act_scale = (_S / _Q) if _USE_INT8 else _S
    sch_a = (_SCH_A / _Q) if _USE_INT8 else _SCH_A
    nc = bacc.Bacc("TRN2", target_bir_lowering=False)
    x = nc.dram_tensor("x", [B * Cs], in_dt, kind="ExternalInput")
    out = nc.dram_tensor(
        "sums", [_P, n_acc + n_d_slots], mybir.dt.float32, kind="ExternalOutput"
    )

    with TileContext(nc) as tc:
        # one buffer per chunk in each engine's input pool: no DMA ever waits
        # on a tile release, so the FIFO Sync queue never head-of-line blocks.
        with (
            tc.tile_pool(name="inA", bufs=max(n_a_chunks, 1)) as inA,
            tc.tile_pool(name="inD", bufs=max(n_d_chunks, 1)) as inD,
            tc.tile_pool(name="scr", bufs=2) as scr,
            tc.tile_pool(name="acc", bufs=1) as accp,
        ):
            bias = accp.tile([_P, 1], mybir.dt.float32)
            nc.gpsimd.memset(bias[:], -_S)
            acc = accp.tile([_P, max(n_acc, 1)], mybir.dt.float32)
            dsl = accp.tile([_P, max(n_d_slots, 1)], mybir.dt.float32)
            ia = [0] * nblk
            idv = [0] * nblk
            off = 0
            for blk, W, eng in plan:
                if eng == "A":
                    t = inA.tile([_P, wmax], in_dt, tag="inA")
                else:
                    t = inD.tile([_P, wmax_d], in_dt, tag="inD")
                nc.sync.dma_start(
                    out=t[:, :W],
                    in_=x[off : off + _P * W].rearrange("(p w) -> p w", p=_P),
                )
                if eng == "A":
                    s = scr.tile([_P, wmax], mybir.dt.float16, tag="scr")
                    j = acc_base[blk] + ia[blk]
                    dst = acc[:, j : j + 1]
                    ia[blk] += 1
                    # s = exp(S*t - S); dst = per-partition row-sum of s
                    nc.scalar.activation(
                        out=s[:, :W],
                        in_=t[:, :W],
                        func=mybir.ActivationFunctionType.Exp,
                        scale=act_scale,
                        bias=bias[:],
                        accum_out=dst,
                    )
                else:
                    assert W % 4 == 0
                    sl = d_base[blk] + idv[blk]
                    idv[blk] += 1
                    i16 = scr.tile([_P, wmax_d], mybir.dt.int16, tag="i16")
                    # int16(A*x + B) bit pattern ~= bf16 exp(S*x - S)
                    nc.vector.tensor_scalar(
                        out=i16[:, :W],
                        in0=t[:, :W],
                        scalar1=sch_a,
                        scalar2=_SCH_B,
                        op0=mybir.AluOpType.mult,
                        op1=mybir.AluOpType.add,
                    )
                    bf = i16[:, :W].bitcast(mybir.dt.bfloat16)
                    h = W // 2
                    q = W // 4
                    f1 = scr.tile([_P, wmax_d // 2], mybir.dt.bfloat16, tag="f1")
                    nc.vector.tensor_tensor(
                        out=f1[:, :h],
                        in0=bf[:, :h],
                        in1=bf[:, h:],
                        op=mybir.AluOpType.add,
                    )
                    f2 = scr.tile([_P, wmax_d // 4], mybir.dt.bfloat16, tag="f2")
                    nc.vector.tensor_tensor(
                        out=f2[:, :q],
                        in0=f1[:, :q],
                        in1=f1[:, q : 2 * q],
                        op=mybir.AluOpType.add,
                    )
                    nc.vector.reduce_sum(
                        out=dsl[:, sl : sl + 1],
                        in_=f2[:, :q],
                        axis=mybir.AxisListType.X,
                    )
                off += _P * W
            # two independent out-DMAs: D slots usually finish first
            nc.sync.dma_start(out=out[:, n_acc:], in_=dsl[:])
            nc.sync.dma_start(out=out[:, :n_acc], in_=acc[:])

    nc.compile()
    return nc


def _get_nc(B, Cs):
    key = (B, Cs)
    if key not in _nc_cache:
        _nc_cache[key] = _build_nc(B, Cs)
    return _nc_cache[key]


def _pack_shard(shard_f16, plan):
    """[B, Cs] fp16 -> flat blob layout matching the global plan DMA order."""
    B, Cs = shard_f16.shape
    nblk = B // _P
    cur = [0] * nblk
    parts = []
    for blk, W, _ in plan:
        rows = shard_f16[blk * _P : (blk + 1) * _P]
        parts.append(rows[:, cur[blk] : cur[blk] + W].ravel())
        cur[blk] += W
    return np.concatenate(parts)


def _device_row_sums(logits, trace=False):
    """Shard the class dim over 8 cores, run the bass kernel, return
    (row_sums[B] float64 = sum_c exp(S*logits - S), BassKernelResults)."""
    from concourse.bass_utils import run_bass_kernel_spmd

    B, C = logits.shape
    Bp = -(-B // _P) * _P  # pad rows to a multiple of 128
    Cp = -(-C // _NCORES) * _NCORES  # pad cols to a multiple of 8
    if _USE_INT8:
        x16 = np.rint(np.maximum(logits, _CLAMP) * _Q).astype(np.int8)
        fill = np.int8(round(_CLAMP * _Q))
    else:
        x16 = np.maximum(logits, _CLAMP).astype(np.float16)
        fill = _CLAMP
    if Bp != B or Cp != C:
        padded = np.full((Bp, Cp), fill, dtype=x16.dtype)
        padded[:B, :C] = x16
        x16 = padded
    Cs = Cp // _NCORES
    nblk = Bp // _P
    plan = _global_plan(nblk, Cs)
    n_dve_by_blk = [sum(1 for b, _, e in plan if b == k and e == "D") for k in range(nblk)]
    d_base = [sum(n_dve_by_blk[:k]) for k in range(nblk)]
    n_act_by_blk = [sum(1 for b, _, e in plan if b == k and e == "A") for k in range(nblk)]
    acc_base = [sum(n_act_by_blk[:k]) for k in range(nblk)]
    n_acc = sum(n_act_by_blk)
    nc = _get_nc(Bp, Cs)
    in_maps = [
        {"x": _pack_shard(x16[:, i * Cs : (i + 1) * Cs], plan)} for i in range(_NCORES)
    ]
    r = run_bass_kernel_spmd(nc, in_maps, core_ids=list(range(_NCORES)), trace=trace)
    total = np.zeros(Bp, np.float64)
    for res in r.results:
        arr = res["sums"].astype(np.float64)  # [128, n_acc + n_d_slots]
        act = np.zeros(Bp)
        dve = np.zeros(Bp)
        for blk in range(nblk):
            rs = slice(blk * _P, (blk + 1) * _P)
            a0 = acc_base[blk]
            act[rs] = arr[:, a0 : a0 + n_act_by_blk[blk]].sum(axis=1)
            lo = n_acc + d_base[blk]
            dve[rs] = arr[:, lo : lo + n_dve_by_blk[blk]].sum(axis=1)
        if _USE_INT8:
            total += _KAPPA * (act + _GAMMA8 * dve)
        else:
            total += act + _GAMMA * dve
    # The clamp floor contributes ~1.8e-35 per clamped element on the ACT
    # side and ~0 on the DVE side; both are below fp32 resolution of the
    # per-row sums (>= exp(0) for a max-logit near 1), so no correction.
    return total[:B], r


def kernel(logits, labels):
    logits = np.ascontiguousarray(np.asarray(logits, dtype=np.float32))
    labels_i = np.asarray(labels).astype(np.int64)
    B, C = logits.shape

    total, _ = _device_row_sums(logits)

    rows = np.arange(B)
    t = logits[rows, labels_i].astype(np.float64)
    # subtract what the device actually added for the label column (its
    # quantized value); the margin math itself uses the exact fp32 target.
    if _USE_INT8:
        t16 = np.rint(np.maximum(t, _CLAMP) * _Q) / _Q
    else:
        t16 = t.astype(np.float16).astype(np.float64)
    thresh = float(np.cos(np.pi - _M2))
    ang = np.arccos(np.clip(t, -1.0 + _EPS, 1.0 - _EPS))
    cos_m = np.cos(ang + _M2)
    theta = np.where(t > thresh, cos_m, -2.0 - cos_m)

    # replace the label column's exp term, all under the constant shift S
    sub = np.exp(_S * t16 - _S)
    if _USE_INT8:
        sub = _KAPPA * sub
    corrected = total - sub + np.exp(_S * theta - _S)
    loss_rows = _S + np.log(corrected) - _S * theta
    return np.array(loss_rows.mean(), dtype=np.float32)
